# revision 1
# baseline (speedup 1.0000x reference)
"""Bass/Tile kernel for causal self-attention, head-sharded across cores.

Per-core layout (core c owns heads 2c, 2c+1):
  inputs (per core):
    xT    [C, B*T]        bf16   x transposed (feature-major), same on all cores
    wqkv  [128, KC, F]    bf16   W_qkv column-slice, [p, kchunk, f]; f = [q_h0|q_h1|k_h0|k_h1|v_h0|v_h1] * 64
    wproj [128, C]        bf16   W_proj row-slice (rows = this core's 128 head dims)
    bqkv  [128, FC]       f32    b_qkv slice, partition-major per f-chunk
    pbias [128, B, T/128] f32    key-padding bias (0 or -1e30), partition-major per key chunk
    masks [128, NDIAG, TB] bf16  0/1 causal masks for diagonal tiles (offset = idx*128)
  output:
    outT  [C, B*T]        f32    partial projection output (pre-bias), feature-major

Phases (emitted interleaved; Tile schedules by dependency):
  1) qkvT[f, t] = W_c^T x  (+bias on DVE)  -> qT/kT tiles [128, TB] bf16, vT staged
     V token-major tiles [128t, 128f] via SBUF->SBUF DMA transpose
  2) per (b, tq-block): for tk chunks <= diag:
       S^T both heads -> one [128, 2*TB] PSUM tile (row-packed concurrent matmuls)
       P = exp(scale*S^T + pbias)  (single ACT op over both heads)
       diagonal-mask multiply (DVE), then col-packed PV pairs:
       V_h0 | V_h1 (concurrent), ones_h0 | ones_h1 (concurrent) accumulating O^T/denom
     A^T = O^T * recip(denom); output projection for the block emitted inline.
"""

import concourse.bass as bass
import concourse.mybir as mybir
import concourse.tile as tile
from concourse import bacc

F32 = mybir.dt.float32
BF16 = mybir.dt.bfloat16
AF = mybir.ActivationFunctionType


def build_nc(B=4, T=2048, C=1024, HPC=2, D=64, TB=512, num_devices=8,
             scale=None, pad_bias=True):
    if scale is None:
        scale = D ** -0.5
    NT = B * T                 # total tokens
    NB = NT // TB              # 512-token blocks (global)
    BPB = T // TB              # blocks per batch
    CPB = TB // 128            # 128-chunks per block (4)
    NCH = T // 128             # key chunks per batch
    KC = C // 128              # contraction chunks for qkv matmul
    F = HPC * 3 * D            # per-core qkv features (384)
    FC = F // 128              # f-chunks (3)
    assert HPC == 2 and HPC * D == 128 and F % 128 == 0 and TB % 128 == 0

    nc = bacc.Bacc("TRN2", target_bir_lowering=False, debug=False,
                   num_devices=num_devices)

    xT = nc.dram_tensor("xT", [C, NT], BF16, kind="ExternalInput")
    wqkv = nc.dram_tensor("wqkv", [128, KC, F], BF16, kind="ExternalInput")
    wproj = nc.dram_tensor("wproj", [128, C], BF16, kind="ExternalInput")
    bqkv = nc.dram_tensor("bqkv", [128, FC], F32, kind="ExternalInput")
    pbias = nc.dram_tensor("pbias", [128, B, NCH], F32, kind="ExternalInput")
    masks = nc.dram_tensor("masks", [128, CPB, TB], BF16, kind="ExternalInput")
    outT = nc.dram_tensor("outT", [C, NT], F32, kind="ExternalOutput")

    with tile.TileContext(nc) as tc:
        with (
            tc.tile_pool(name="const", bufs=1) as const,
            tc.tile_pool(name="persist", bufs=1) as persist,
            tc.tile_pool(name="xp", bufs=16) as xp,
            tc.tile_pool(name="pp", bufs=10) as pp,
            tc.tile_pool(name="rp", bufs=2) as rp,
            tc.tile_pool(name="op", bufs=10) as op,
            tc.tile_pool(name="psmm", bufs=4, space="PSUM") as psmm,
            tc.tile_pool(name="pss", bufs=2, space="PSUM") as pss,
        ):
            # ---- constants ----
            w_sb = const.tile([128, KC, F], BF16, tag="w", name="w_sb")
            nc.gpsimd.dma_start(out=w_sb[:], in_=wqkv[:])
            wp_sb = const.tile([128, C], BF16, tag="wp", name="wp_sb")
            nc.gpsimd.dma_start(out=wp_sb[:], in_=wproj[:])
            bq_sb = const.tile([128, FC], F32, tag="bq", name="bq_sb")
            nc.gpsimd.dma_start(out=bq_sb[:], in_=bqkv[:])
            pb_sb = const.tile([128, B, NCH], F32, tag="pb", name="pb_sb")
            nc.gpsimd.dma_start(out=pb_sb[:], in_=pbias[:])
            # mask replicated for both heads: [128, CPB, 2*TB]
            mk_sb = const.tile([128, CPB, 2 * TB], BF16, tag="mk", name="mk_sb")
            for h in range(HPC):
                nc.gpsimd.dma_start(
                    out=mk_sb.rearrange("p c (h t) -> p c h t", h=2)[:, :, h, :],
                    in_=masks[:])
            ones_sb = const.tile([128, 64], BF16, tag="ones", name="ones_sb")
            nc.vector.memset(ones_sb[:], 1.0)

            # ---- persistent per-block tiles ----
            qT = [persist.tile([128, TB], BF16, tag=f"qT{i}", name=f"qT{i}")
                  for i in range(NB)]
            kT = [persist.tile([128, TB], BF16, tag=f"kT{i}", name=f"kT{i}")
                  for i in range(NB)]
            # token-major V (2 heads stacked in free dim), one per 512-token
            # block: [128 t, CPB chunk, 128 f] filled by one batched transpose
            V = [persist.tile([128, CPB, 128], BF16, tag=f"V{i}", name=f"V{i}")
                 for i in range(NB)]

            # ---- phase 1: QKV projection ----
            # x loaded in groups of GRP t-blocks: wide DMAs keep 16 engines fed
            GRP = 4
            assert NB % GRP == 0

            def qkv_group(g):
                x_tiles = []
                W = GRP * TB
                for kc in range(KC):
                    xt = xp.tile([128, W], BF16, tag="xt", name="xt")
                    # two half-loads on different DGE paths -> more DMA engines
                    nc.gpsimd.dma_start(
                        out=xt[:, 0:W // 2],
                        in_=xT[kc * 128:(kc + 1) * 128,
                               g * W:g * W + W // 2])
                    nc.sync.dma_start(
                        out=xt[:, W // 2:W],
                        in_=xT[kc * 128:(kc + 1) * 128,
                               g * W + W // 2:(g + 1) * W])
                    x_tiles.append(xt)
                for tl in range(GRP):
                    tb = g * GRP + tl
                    for fc in range(FC):
                        ps = psmm.tile([128, TB], F32, tag="ps", name="ps")
                        for kc in range(KC):
                            nc.tensor.matmul(
                                ps[:], lhsT=w_sb[:, kc, fc * 128:(fc + 1) * 128],
                                rhs=x_tiles[kc][:, tl * TB:(tl + 1) * TB],
                                start=(kc == 0), stop=(kc == KC - 1))
                        if fc == 0:
                            dest = qT[tb]
                        elif fc == 1:
                            dest = kT[tb]
                        else:
                            dest = persist.tile([128, TB], BF16,
                                                tag=f"vs{tb % 2}", name="vs")
                        # bias-add + cast on DVE (ACT is the busier engine
                        # once QKV overlaps attention exps)
                        nc.vector.tensor_scalar_add(
                            out=dest[:], in0=ps[:], scalar1=bq_sb[:, fc:fc + 1])
                        if fc == 2:
                            nc.sync.dma_start_transpose(
                                out=V[tb][:], in_=dest[:])

            # ---- phase 2+3: attention + inline projection ----
            def attn_block(b, qb):
                gb = b * BPB + qb
                nchunks = (qb + 1) * CPB
                psO = psmm.tile([128, TB], F32, tag="ps", name="psO")
                psD = psmm.tile([128, TB], F32, tag="ps", name="psD")
                assert nchunks % 2 == 0
                for c0 in range(0, nchunks, 2):
                    pts = []
                    for ci in (c0, c0 + 1):
                        cb = ci // CPB      # kT block within batch
                        cl = ci % CPB       # 128-chunk within that block
                        ktile = kT[b * BPB + cb]
                        # S^T for both heads into one 2-bank PSUM tile
                        psS = pss.tile([128, 2 * TB], F32, tag="pss", name="psS")
                        for h in range(HPC):
                            nc.tensor.matmul(
                                psS[:, h * TB:(h + 1) * TB],
                                lhsT=ktile[h * 64:(h + 1) * 64,
                                           cl * 128:(cl + 1) * 128],
                                rhs=qT[gb][h * 64:(h + 1) * 64, :],
                                start=True, stop=True)
                        pt = pp.tile([128, 2 * TB], BF16, tag="pt", name="pt")
                        if pad_bias:
                            nc.scalar.activation(
                                out=pt[:], in_=psS[:], func=AF.Exp,
                                bias=pb_sb[:, b, ci:ci + 1], scale=scale)
                        else:
                            nc.scalar.activation(
                                out=pt[:], in_=psS[:], func=AF.Exp, scale=scale)
                        if ci >= qb * CPB:  # diagonal tile: causal mask
                            offidx = ci - qb * CPB
                            nc.vector.tensor_mul(
                                pt[:], pt[:], mk_sb[:, offidx, :])
                        pts.append(pt)
                    # all PV matmuls for both chunks, then all denominator
                    # matmuls: consecutive pair-slots with no new waits between
                    for j, ci in enumerate((c0, c0 + 1)):
                        vtile = V[b * BPB + ci // CPB]
                        for h in range(HPC):
                            nc.tensor.matmul(
                                psO[h * 64:(h + 1) * 64, :],
                                lhsT=vtile[:, ci % CPB, h * 64:(h + 1) * 64],
                                rhs=pts[j][:, h * TB:(h + 1) * TB],
                                start=(ci == 0), stop=(ci == nchunks - 1),
                                tile_position=(0, h * 64))
                    for j, ci in enumerate((c0, c0 + 1)):
                        for h in range(HPC):
                            nc.tensor.matmul(
                                psD[h * 64:(h + 1) * 64, :],
                                lhsT=ones_sb[:],
                                rhs=pts[j][:, h * TB:(h + 1) * TB],
                                start=(ci == 0), stop=(ci == nchunks - 1),
                                tile_position=(0, h * 64))
                # normalize: A^T = O^T * (1/denom)
                rt = rp.tile([128, TB], F32, tag="rt", name="rt")
                nc.vector.reciprocal_approx_fast(out=rt[:], in_=psD[:])
                at = pp.tile([128, TB], BF16, tag="at", name="at")
                nc.vector.tensor_mul(at[:], psO[:], rt[:])
                # inline output projection for this block
                for fc in range(C // 128):
                    ps = psmm.tile([128, TB], F32, tag="ps", name="ps")
                    nc.tensor.matmul(ps[:],
                                     lhsT=wp_sb[:, fc * 128:(fc + 1) * 128],
                                     rhs=at[:], start=True, stop=True)
                    ot = op.tile([128, TB], F32, tag="ot", name="ot")
                    nc.vector.tensor_copy(ot[:], ps[:])
                    nc.gpsimd.dma_start(
                        out=outT[fc * 128:(fc + 1) * 128,
                                 gb * TB:(gb + 1) * TB],
                        in_=ot[:])

            # interleave emission: QKV group g produces batch g's blocks
            # (GRP == BPB), so attention for batch b can weave between later
            # groups -- fills PE during DMA-bound stretches of QKV.
            ngroups = NB // GRP
            emitted = 0
            qkv_group(0)
            for g in range(1, ngroups):
                qkv_group(g)
                if g >= 2:
                    b = emitted
                    for qb in range(BPB):
                        attn_block(b, qb)
                    emitted += 1
            for b in range(emitted, B):
                for qb in range(BPB):
                    attn_block(b, qb)

    nc.compile()
    return nc


def prep_core_inputs(x, key_padding_mask, W_qkv, b_qkv, W_proj,
                     n_cores=8, TB=512):
    """Host-side sharding: build the per-core input maps."""
    import numpy as np
    import ml_dtypes

    B, T, C = x.shape
    D = 64
    H = C // D
    HPC = H // n_cores
    BT = B * T
    CPB = TB // 128

    xT = np.ascontiguousarray(
        x.reshape(BT, C).T).astype(ml_dtypes.bfloat16)          # [C, BT]

    pb = np.where(key_padding_mask, np.float32(-1e30),
                  np.float32(0.0)).astype(np.float32)           # [B, T]
    pb = np.ascontiguousarray(pb.reshape(B, T // 128, 128).transpose(2, 0, 1))

    p = np.arange(128)[:, None]
    j = np.arange(TB)[None, :]
    mk = np.stack([(o * 128 + p <= j) for o in range(CPB)], axis=1)
    mk = mk.astype(ml_dtypes.bfloat16)                          # [128, CPB, TB]

    KC = C // 128
    in_maps = []
    for c in range(n_cores):
        hs = [HPC * c + i for i in range(HPC)]
        cols = np.concatenate([
            np.concatenate([which * H * D + h * D + np.arange(D) for h in hs])
            for which in range(3)])                             # [F]
        Wc = W_qkv[:, cols]                                     # [C, F]
        F = Wc.shape[1]
        wq = np.ascontiguousarray(
            Wc.reshape(KC, 128, F).transpose(1, 0, 2)).astype(ml_dtypes.bfloat16)
        bq = np.ascontiguousarray(
            b_qkv[cols].reshape(F // 128, 128).T).astype(np.float32)
        rows = np.concatenate([h * D + np.arange(D) for h in hs])
        wp = np.ascontiguousarray(W_proj[rows, :]).astype(ml_dtypes.bfloat16)
        in_maps.append({
            "xT": xT, "wqkv": wq.reshape(128, KC, F), "wproj": wp,
            "bqkv": bq, "pbias": pb, "masks": mk,
        })
    return in_maps


def combine_outputs(results, B, T, C, b_proj):
    import numpy as np
    acc = results[0]["outT"].astype(np.float32)
    for r in results[1:]:
        acc = acc + r["outT"]
    out = acc.T.reshape(B, T, C) + b_proj.astype(np.float32)
    return out.astype(np.float32)


# ---------------------------------------------------------------------------
# Self-contained entry point for the grading harness.
# kernel(**inputs) takes the FULL unsharded inputs and returns the FULL output.
# Sharding: tensor-parallel over heads (2 heads per core, 8 cores); each core
# computes its QKV column-slice, attention for its heads, and a partial output
# projection; partials are summed on the host.
# ---------------------------------------------------------------------------
import numpy as np

_NC_CACHE = {}


def _get_nc():
    if "nc" not in _NC_CACHE:
        _NC_CACHE["nc"] = build_nc(B=4, T=2048, C=1024, num_devices=8)
    return _NC_CACHE["nc"]


def kernel(x, key_padding_mask, W_qkv, b_qkv, W_proj, b_proj):
    from concourse.bass_utils import run_bass_kernel_spmd

    x = np.asarray(x, dtype=np.float32)
    key_padding_mask = np.asarray(key_padding_mask).astype(bool)
    W_qkv = np.asarray(W_qkv, dtype=np.float32)
    b_qkv = np.asarray(b_qkv, dtype=np.float32)
    W_proj = np.asarray(W_proj, dtype=np.float32)
    b_proj = np.asarray(b_proj, dtype=np.float32)

    B, T, C = x.shape
    nc = _get_nc()
    in_maps = prep_core_inputs(x, key_padding_mask, W_qkv, b_qkv, W_proj,
                               n_cores=8)
    res = run_bass_kernel_spmd(nc, in_maps, list(range(8)))
    return combine_outputs(res.results, B, T, C, b_proj)

